# revision 4
# baseline (speedup 1.0000x reference)
"""Causal multi-head attention block on 8 Trainium2 NeuronCores.

Sharding: data parallel over batch (B == 8 == n_cores); each core runs one
batch element with full weights, no collectives.

Design (per core), v2 "unified stream":
  - host packs every input into SBUF-shaped bf16 blobs so each DMA is a
    simple contiguous per-partition transfer (cheap descriptor generation);
    all input DMAs issue on the sync queue before any compute is emitted.
  - all matmul phases (B2 = q/k projection, B1 = v projection, attention,
    output projection) are chopped into ~1us "chunks" and woven into ONE
    PE instruction stream: attention pair-units are interleaved with
    projection chunks so the tensor engine never waits on the scalar
    engine's softmax exp (exp time == attention-PE time, so pure-attention
    stretches are chain-limited; fillers hide that).
  - engine balance: exp + q-copies on scalar, k-copies/v-adds/out-adds/
    epilogue muls on DVE, diagonal causal masks + denominator broadcasts
    on gpsimd.
  - proj is slab-split: m-tiles 0-3 (rows i<512) only need slab-0 of all
    heads and run as fillers inside the last attention group; only
    m-tiles 4-7 trail the last attention unit.
  - softmax denominators via a ones-column in v_aug (PV matmul emits them
    as PSUM row 64); per head-pair the two denominator rows are packed
    into one [2,512] tile so one reciprocal_approx_fast serves both heads.
"""

import numpy as np
import ml_dtypes
from contextlib import ExitStack

import concourse.bass as bass
import concourse.mybir as mybir
import concourse.tile as tile
from concourse import bacc
from concourse.bass_utils import run_bass_kernel_spmd
from concourse.masks import make_identity

F32 = mybir.dt.float32
BF16 = mybir.dt.bfloat16
AF = mybir.ActivationFunctionType

B, T, C = 8, 1024, 768
H, HS = 12, 64
KT = C // 128            # 6 contraction tiles
MT = T // 128            # 8 row tiles (also j tiles)
NPAIR = 6                # head pairs
SCALE = 1.0 / np.sqrt(HS)
N_WARMUP = 48

N_CORES = 8


def build_program():
    nc = bacc.Bacc("TRN2", target_bir_lowering=False, debug=False)

    xt_d = nc.dram_tensor("xt", [128, KT, T], BF16, kind="ExternalInput")
    wqk_d = [nc.dram_tensor(f"wqk{p}", [128, KT, 256], BF16, kind="ExternalInput")
             for p in range(NPAIR)]
    wv0_d = nc.dram_tensor("wv0", [128, KT, 512], BF16, kind="ExternalInput")
    wv1_d = nc.dram_tensor("wv1", [128, KT, 256], BF16, kind="ExternalInput")
    wp_d = nc.dram_tensor("wp", [128, KT, C], BF16, kind="ExternalInput")
    bias_d = nc.dram_tensor("biasb", [128, 18 + 2 * C], F32, kind="ExternalInput")
    out_d = nc.dram_tensor("out", [T, C], F32, kind="ExternalOutput")

    with tile.TileContext(nc) as tc, ExitStack() as ctx:
        consts = ctx.enter_context(tc.tile_pool(name="consts", bufs=1))
        big = ctx.enter_context(tc.tile_pool(name="big", bufs=1))
        io = ctx.enter_context(tc.tile_pool(name="io", bufs=2))
        pt_pool = ctx.enter_context(tc.tile_pool(name="pt", bufs=4))
        rl_pool = ctx.enter_context(tc.tile_pool(name="rl", bufs=3))
        ps_st = ctx.enter_context(tc.tile_pool(name="ps_st", bufs=2, space="PSUM"))
        ps_y = ctx.enter_context(tc.tile_pool(name="ps_y", bufs=2, space="PSUM"))
        ps_big = ctx.enter_context(tc.tile_pool(name="ps_big", bufs=2, space="PSUM"))

        # ---- input DMAs first, sync queue, consumption order ----
        biasb = consts.tile([128, 18 + 2 * C], F32, tag="biasb")
        nc.sync.dma_start(out=biasb, in_=bias_d[:])
        xT = big.tile([128, KT, T], BF16, tag="xT")
        nc.sync.dma_start(out=xT, in_=xt_d[:])
        wqk = big.tile([128, NPAIR, KT, 256], BF16, tag="wqk")
        nc.sync.dma_start(out=wqk[:, 0], in_=wqk_d[0][:])
        wv0 = big.tile([128, KT, 512], BF16, tag="wv0")
        nc.sync.dma_start(out=wv0, in_=wv0_d[:])
        for p in range(1, NPAIR):
            nc.sync.dma_start(out=wqk[:, p], in_=wqk_d[p][:])
        wv1 = big.tile([128, KT, 256], BF16, tag="wv1")
        nc.sync.dma_start(out=wv1, in_=wv1_d[:])
        wpr = big.tile([128, KT, C], BF16, tag="wp")
        nc.sync.dma_start(out=wpr, in_=wp_d[:])

        battn_t = biasb[:, 0:18]
        bv_b = biasb[:, 18:18 + C]
        bp_b = biasb[:, 18 + C:18 + 2 * C]

        # ---- constants ----
        ident16 = consts.tile([128, 128], BF16, tag="ident16")
        make_identity(nc, ident16)
        ZR = nc.gpsimd.to_reg(0.0)

        # ---- PE warm-up while the DMAs land ----
        for _ in range(N_WARMUP):
            ps = ps_big.tile([128, 512], F32, tag="mm")
            nc.tensor.matmul(ps[:, 0:128], ident16, ident16, start=True, stop=True)

        # ---- SBUF working tensors ----
        qk16 = big.tile([128, 12, T], BF16, tag="qk16")
        v_aug = big.tile([128, MT, H, HS + 1], BF16, tag="v_aug")
        nc.gpsimd.memset(v_aug[:, :, :, HS:HS + 1], 1.0)
        yT16 = big.tile([128, KT, T], BF16, tag="yT16")

        # ---- chunk emitters (each ~0.6-1.3us of PE work) ----
        def b2_chunk(p, x, mc):
            # q/k projection for head-pair p; x: 0=q, 1=k; mc: T-half
            nt = p if x == 0 else 6 + p
            ps = ps_big.tile([128, 512], F32, tag="mm")
            for kt in range(KT):
                nc.tensor.matmul(
                    ps, wqk[:, p, kt, x * 128:(x + 1) * 128],
                    xT[:, kt, mc * 512:(mc + 1) * 512],
                    start=(kt == 0), stop=(kt == KT - 1),
                )
            dst = qk16[:, nt, mc * 512:(mc + 1) * 512]
            if x == 0:
                nc.scalar.activation(dst, ps, AF.Identity,
                                     bias=battn_t[:, nt:nt + 1], scale=1.0)
            else:
                nc.vector.tensor_scalar_add(dst, ps, battn_t[:, nt:nt + 1])

        def b1_chunk(c, mt):
            # v projection: c=0 -> heads 0-7 (cols 0:512), c=1 -> heads 8-11
            n0, nsz, wv = (0, 512, wv0) if c == 0 else (512, 256, wv1)
            ps = ps_big.tile([128, 512], F32, tag="mm")
            for kt in range(KT):
                nc.tensor.matmul(
                    ps[:, 0:nsz], xT[:, kt, mt * 128:(mt + 1) * 128],
                    wv[:, kt, 0:nsz],
                    start=(kt == 0), stop=(kt == KT - 1),
                )
            h0, nh = n0 // HS, nsz // HS
            nc.vector.tensor_add(
                v_aug[:, mt, h0:h0 + nh, 0:HS],
                ps[:, 0:nsz].rearrange("p (h d) -> p h d", d=HS),
                bv_b[:, n0:n0 + nsz].rearrange("p (h d) -> p h d", d=HS),
            )

        def proj_chunk(mt, cc, state):
            c0, csz = (0, 512) if cc == 0 else (512, 256)
            if cc == 0:
                state["sb"] = io.tile([128, C], F32, tag="io",
                                      name=f"out_sb_{mt}")
            out_sb = state["sb"]
            ps = ps_big.tile([128, 512], F32, tag="mm")
            for nt in range(KT):
                nc.tensor.matmul(
                    ps[:, 0:csz], yT16[:, nt, mt * 128:(mt + 1) * 128],
                    wpr[:, nt, c0:c0 + csz],
                    start=(nt == 0), stop=(nt == KT - 1),
                )
            nc.vector.tensor_add(
                out_sb[:, c0:c0 + csz], ps[:, 0:csz], bp_b[:, c0:c0 + csz])
            if cc == 1:
                nc.sync.dma_start(
                    out=out_d[mt * 128:(mt + 1) * 128, :], in_=out_sb)

        def proj_fillers(mts):
            fl = []
            for mt in mts:
                state = {}
                fl.append(lambda mt=mt, state=state: proj_chunk(mt, 0, state))
                fl.append(lambda mt=mt, state=state: proj_chunk(mt, 1, state))
            return fl

        # ---- attention group: two heads (h0, h0+1), one 512-wide i-slab ----
        def attn_group(h0, slab, fillers):
            heads = []
            for h in (h0, h0 + 1):
                nt_q, po = h // 2, 64 * (h % 2)
                heads.append((h, nt_q, po,
                              qk16[po:po + 64, nt_q, :],
                              qk16[po:po + 64, 6 + nt_q, :]))
            i0 = 512 * slab
            njt = 4 * (slab + 1)
            pairs = []
            for jp in range(njt // 2):
                jtA, jtB = 2 * jp, 2 * jp + 1
                nA = i0 + 512 - max(i0, jtA * 128)
                nB = i0 + 512 - max(i0, jtB * 128)
                offB = 512 if nA == 512 else nA
                pairs.append((jtA, jtB, nA, nB, offB))

            yas = {h: ps_y.tile([HS + 1, 512], F32, tag="ya",
                                name=f"ya_{h}_{slab}")
                   for h, _, _, _, _ in heads}

            def emit_qk_pair(hh, pr):
                h, nt_q, po, qT_h, kT_h = hh
                jtA, jtB, nA, nB, offB = pr
                st2 = ps_st.tile([128, 1024], F32, tag="st2")
                for jt, off, n in ((jtA, 0, nA), (jtB, offB, nB)):
                    nc.tensor.matmul(
                        st2[:, off:off + n],
                        kT_h[:, jt * 128:(jt + 1) * 128],
                        qT_h[:, i0 + 512 - n:i0 + 512],
                        start=True, stop=True,
                    )
                pt2 = pt_pool.tile([128, 1024], BF16, tag="ptile")
                nc.scalar.activation(
                    pt2[:, 0:offB + nB], st2[:, 0:offB + nB],
                    AF.Exp, bias=0.0, scale=SCALE)
                for jt, off in ((jtA, 0), (jtB, offB)):
                    if jt * 128 >= i0:  # diagonal block: zero j > i
                        nc.gpsimd.affine_select(
                            out=pt2[:, off:off + 128],
                            in_=pt2[:, off:off + 128],
                            compare_op=mybir.AluOpType.is_ge, fill=ZR,
                            base=0, pattern=[[1, 128]], channel_multiplier=-1,
                        )
                return (hh, pr, pt2)

            def emit_pv(item):
                (h, nt_q, po, qT_h, kT_h), pr, pt2 = item
                jtA, jtB, nA, nB, offB = pr
                for jt, off, n in ((jtA, 0, nA), (jtB, offB, nB)):
                    nc.tensor.matmul(
                        yas[h][:, 512 - n:512], v_aug[:, jt, h, :],
                        pt2[:, off:off + n],
                        start=(jt == 0), stop=(jt == njt - 1),
                    )

            stage = []
            fi = 0
            for pr in pairs:
                for hh in heads:
                    stage.append(emit_qk_pair(hh, pr))
                    if fi < len(fillers):
                        fillers[fi]()
                        fi += 1
                    if len(stage) > 2:
                        emit_pv(stage.pop(0))
            while stage:
                emit_pv(stage.pop(0))

            # epilogue: denominator reciprocal + broadcast + normalize
            for h, nt_q, po, _, _ in heads:
                lrow = rl_pool.tile([1, 512], F32, tag="lrow")
                nc.vector.tensor_copy(lrow, yas[h][HS:HS + 1, :])
                rl = rl_pool.tile([1, 512], F32, tag="rl")
                nc.vector.reciprocal_approx_fast(rl, lrow)
                rlb = rl_pool.tile([64, 512], F32, tag="rlb")
                nc.gpsimd.partition_broadcast(rlb, rl)
                nc.vector.tensor_mul(
                    yT16[po:po + 64, nt_q, i0:i0 + 512],
                    yas[h][0:HS, :], rlb)

            # leftover fillers (after epilogue so the DVE queue serves it first)
            while fi < len(fillers):
                fillers[fi]()
                fi += 1

        # ---- phase I: q/k for pair 0, v for heads 0-7 (covers ingest) ----
        for mc in range(2):
            b2_chunk(0, 0, mc)
            b2_chunk(0, 1, mc)
        for mt in range(MT):
            b1_chunk(0, mt)

        # ---- unified stream: pair-major, slab0 then slab1, with fillers ----
        def b2_fillers(p):
            return [lambda mc=mc, x=x: b2_chunk(p, x, mc)
                    for mc in range(2) for x in range(2)]

        b1c1 = [lambda mt=mt: b1_chunk(1, mt) for mt in range(MT)]

        attn_group(0, 0, b2_fillers(1))
        attn_group(0, 1, b2_fillers(2) + b1c1[0:2])
        attn_group(2, 0, b1c1[2:4])
        attn_group(2, 1, b2_fillers(3) + b1c1[4:6])
        attn_group(4, 0, b1c1[6:8])
        attn_group(4, 1, b2_fillers(4))
        attn_group(6, 0, [])
        attn_group(6, 1, b2_fillers(5))
        attn_group(8, 0, [])
        attn_group(8, 1, [])
        attn_group(10, 0, [])
        attn_group(10, 1, proj_fillers(range(0, 4)))

        # ---- tail: remaining output-projection tiles ----
        for fl in proj_fillers(range(4, MT)):
            fl()

    nc.compile()
    return nc


_CACHE = {}


def _get_program():
    if "nc" not in _CACHE:
        _CACHE["nc"] = build_program()
    return _CACHE["nc"]


def _prep_host_inputs(x, w_attn, b_attn, w_proj, b_proj):
    BF = ml_dtypes.bfloat16
    x = np.asarray(x, np.float32)            # [B, T, C]
    w = np.asarray(w_attn, np.float32)       # [C, 3C]
    wp = np.asarray(w_proj, np.float32)      # [C, C]
    ba = np.asarray(b_attn, np.float32)      # [3C]
    bp = np.asarray(b_proj, np.float32)      # [C]

    xt = x.transpose(0, 2, 1).reshape(B, KT, 128, T).transpose(0, 2, 1, 3)
    xt16 = np.ascontiguousarray(xt).astype(BF)              # [B, 128, KT, T]

    wq = w[:, 0:C].reshape(C, NPAIR, 128)
    wk = w[:, C:2 * C].reshape(C, NPAIR, 128)
    wqk = np.stack([wq, wk], axis=2)                        # [C, 6, 2, 128]
    wqk = wqk.reshape(KT, 128, NPAIR, 256).transpose(1, 2, 0, 3)
    wqk16 = [np.ascontiguousarray(wqk[:, p]).astype(BF) for p in range(NPAIR)]

    wv = w[:, 2 * C:3 * C]
    wv0 = np.ascontiguousarray(
        wv[:, 0:512].reshape(KT, 128, 512).transpose(1, 0, 2)).astype(BF)
    wv1 = np.ascontiguousarray(
        wv[:, 512:768].reshape(KT, 128, 256).transpose(1, 0, 2)).astype(BF)
    wp16 = np.ascontiguousarray(
        wp.reshape(KT, 128, C).transpose(1, 0, 2)).astype(BF)

    battn_t = ba.reshape(18, 128).T                         # [128, 18]
    bias_blob = np.ascontiguousarray(np.concatenate(
        [battn_t,
         np.broadcast_to(ba[2 * C:3 * C], (128, C)),
         np.broadcast_to(bp, (128, C))], axis=1)).astype(np.float32)

    base = {f"wqk{p}": wqk16[p] for p in range(NPAIR)}
    base.update({"wv0": wv0, "wv1": wv1, "wp": wp16, "biasb": bias_blob})
    return [dict(base, xt=np.ascontiguousarray(xt16[b])) for b in range(B)]


def kernel(x, w_attn, b_attn, w_proj, b_proj):
    nc = _get_program()
    in_maps = _prep_host_inputs(x, w_attn, b_attn, w_proj, b_proj)
    res = run_bass_kernel_spmd(nc, in_maps, list(range(N_CORES)))
    return np.stack([res.results[b]["out"] for b in range(B)], axis=0)
